# revision 1
# baseline (speedup 1.0000x reference)
"""Distributed Trainium2 Bass kernel for a 2-layer ChebConv GCN (K=4).

Strategy (8 NeuronCores, node sharding):
  - Nodes padded to 50176 = 8 shards x 6272 (49 blocks of 128).
  - The gather table Y_k = dinv * Tx_k (node-major bf16 rows, 256B) is
    replicated in every core's HBM; per Chebyshev hop each core
    dma_gather's the source rows of its destination-sharded edges,
    reduces them per 128-dst block with one-hot matmuls on the
    TensorEngine (one-hot built on the VectorEngine from edge metadata),
    and applies the recurrence Tx_{k+1} = sc*u - Tx_{k-1} on PSUM exit.
  - Halo exchange = AllGather of each core's recomputed shard.
  - Edges are bucketed by (dst shard, dst block) on the host and split
    lo/hi on src < 32768 (int16 gather index limit); tile counts are
    equalized across cores so all 8 cores run one SPMD program.
  - Dense GEMMs (x@W per hop), BN (stats AllReduce), ReLU, and the
    linear head run on-device, feature-major, fused into the rounds.
"""
import sys

sys.path.insert(0, "/opt/trn_rl_repo")

import numpy as np
import ml_dtypes

import concourse.bass as bass
import concourse.bacc as bacc
import concourse.tile as tile
import concourse.mybir as mybir
from concourse import library_config
from concourse.bass_utils import run_bass_kernel_spmd

BF16 = mybir.dt.bfloat16
F32 = mybir.dt.float32
I16 = mybir.dt.int16
OP = mybir.AluOpType

N = 50000
E = 800000
NCORE = 8
BLK = 128
NBLK = 49                 # blocks per shard
SH = NBLK * BLK           # 6272
NPAD = NCORE * SH         # 50176
NBG = NPAD // BLK         # 392 global blocks
LO_LIM = 32768
F = 128                   # feature width of both cheb layers' inputs
CL1 = 128
CL2 = 64
OUT_F = 10
K = 4
BN_EPS = 1e-5

import os as _os


def _env(name, default):
    return int(_os.environ.get("CHEBK_" + name, default))


GCH = _env("GCH", "8")     # gather chunk, tiles of 128 edges
NOAG = _env("NOAG", "0")
NOMM = _env("NOMM", "0")
NOGATHER = _env("NOGATHER", "0")
NOMGEN = _env("NOMGEN", "0")
NOEPI = _env("NOEPI", "0")
NOGEMM = _env("NOGEMM", "0")
MB = _env("MB", "8")       # M-gen batch, tiles per DVE instr
GBUFS = _env("GBUFS", "4")
MBUFS = _env("MBUFS", "2")
PABUFS = _env("PABUFS", "3")
PTBUFS = _env("PTBUFS", "1")
YSACT = _env("YSACT", "1")


# ----------------------------------------------------------------- host prep
def _preprocess(edge_index):
    src = np.asarray(edge_index[0], dtype=np.int64)
    dst = np.asarray(edge_index[1], dtype=np.int64)

    deg = np.bincount(src, minlength=NPAD).astype(np.float64)
    dinv = np.zeros(NPAD, np.float32)
    nz = deg > 0
    dinv[nz] = (1.0 / np.sqrt(deg[nz])).astype(np.float32)

    shard = dst // SH
    block = (dst % SH) // BLK
    ld = (dst % BLK).astype(np.int16)
    is_hi = (src >= LO_LIM).astype(np.int64)

    # order edges by (core, part, block)
    key = ((shard * 2 + is_hi) * NBLK + block).astype(np.int64)
    order = np.argsort(key, kind="stable")
    counts = np.bincount(key, minlength=NCORE * 2 * NBLK).reshape(
        NCORE, 2, NBLK
    )

    T = np.maximum(1, -(-counts.max(axis=0) // BLK))  # [2, NBLK]

    # per-core slot streams
    starts = np.zeros(NCORE * 2 * NBLK + 1, np.int64)
    np.cumsum(counts.reshape(-1), out=starts[1:])
    idx_s, ld_s = src[order], ld[order]

    streams = []
    tot = {0: int(T[0].sum()) * BLK, 1: int(T[1].sum()) * BLK}
    for c in range(NCORE):
        out = {}
        for p in range(2):
            ii = np.zeros(tot[p], np.int16)
            dd = np.full(tot[p], -1, np.int16)
            off = 0
            for b in range(NBLK):
                k = (c * 2 + p) * NBLK + b
                s0, s1 = starts[k], starts[k + 1]
                n = s1 - s0
                base = 0 if p == 0 else LO_LIM
                ii[off : off + n] = (idx_s[s0:s1] - base).astype(np.int16)
                dd[off : off + n] = ld_s[s0:s1]
                off += T[p, b] * BLK
            out[p] = (ii, dd)
        streams.append(out)
    return dinv, T, streams


def _wrap_idx16(a):
    return np.tile(a.reshape(-1, 16).T, (8, 1)).copy()


def _ldst_tiles(a):
    nt = a.size // BLK
    return a.reshape(nt, BLK).T.astype(np.float32).copy()


def _ldst_tiles_f(a):
    nt = a.size // BLK
    return np.ascontiguousarray(
        a.reshape(nt, BLK).T.astype(np.float32)
    )


# ------------------------------------------------------------- program build
def _build(T, NR1=3, NR2=3, PH=9):
    TLO, THI = int(T[0].sum()), int(T[1].sum())
    SLO, SHI = TLO * BLK, THI * BLK

    nc = bacc.Bacc(
        "TRN2", target_bir_lowering=False, debug=False, num_devices=NCORE
    )

    def din(name, shape, dt):
        return nc.dram_tensor(name, shape, dt, kind="ExternalInput")

    xbf = din("xbf", [NPAD, F], BF16)
    xsh = din("xsh", [SH, F], F32)
    idx_lo = din("idx_lo", [128, SLO // 16], I16)
    idx_hi = din("idx_hi", [128, SHI // 16], I16)
    ldst_lo = din("ldst_lo", [128, TLO], F32)
    ldst_hi = din("ldst_hi", [128, THI], F32)
    dsl_in = din("dsl", [128, TLO], F32)
    dsh_in = din("dsh", [128, THI], F32)
    sc_k0 = din("sc_k0", [128, NBLK], F32)   # -dinv (shard)
    sc_k = din("sc_k", [128, NBLK], F32)     # -2*dinv (shard)
    dinv_sh = din("dinv_sh", [128, NBLK], F32)
    w1 = din("w1", [128, K * CL1], BF16)
    w2 = din("w2", [128, K * CL2], BF16)
    wl = din("wl", [CL2, OUT_F], F32)
    b1 = din("b1", [128, 1], F32)
    b2 = din("b2", [CL2, 1], F32)
    bl = din("bl", [OUT_F, 1], F32)
    gamma = din("gamma", [128, 1], F32)
    beta = din("beta", [128, 1], F32)
    iota_in = din("iota", [128, 128], BF16)
    eye_in = din("eye", [128, 128], BF16)

    out_t = nc.dram_tensor("out", [OUT_F, SH], F32, kind="ExternalOutput")

    # internal dram
    tbls = [xbf] + [
        nc.dram_tensor(f"tbl{r}", [NPAD, F], BF16, addr_space="Shared")
        for r in range(1, 6)
    ]
    agins = {
        r: nc.dram_tensor(f"agin{r}", [SH, F], BF16) for r in (1, 2, 3, 4, 5)
    }
    bnc_in = nc.dram_tensor("bnc_in", [128, 2], F32)
    bnc_out = nc.dram_tensor("bnc_out", [128, 2], F32, addr_space="Shared")

    # persistent SBUF
    sb = lambda n, s, d: nc.alloc_sbuf_tensor(n, s, d)
    idx_lo_sb = sb("idx_lo_sb", [128, SLO // 16], I16)
    idx_hi_sb = sb("idx_hi_sb", [128, SHI // 16], I16)
    ldst_lo_sb = sb("ldst_lo_sb", [128, TLO], F32)
    ldst_hi_sb = sb("ldst_hi_sb", [128, THI], F32)
    dsl_sb = sb("dsl_sb", [128, TLO], F32)
    dsh_sb = sb("dsh_sb", [128, THI], F32)
    iota_sb = sb("iota_sb", [128, 128], BF16)
    eye_sb = sb("eye_sb", [128, 128], BF16)
    w1_sb = sb("w1_sb", [128, K * CL1], BF16)
    w2_sb = sb("w2_sb", [128, K * CL2], BF16)
    wl_sb = sb("wl_sb", [CL2, OUT_F], F32)
    sc_k0_sb = sb("sc_k0_sb", [128, NBLK], F32)
    sc_k_sb = sb("sc_k_sb", [128, NBLK], F32)
    dinv_sh_sb = sb("dinv_sh_sb", [128, NBLK], F32)
    b1_sb = sb("b1_sb", [128, 1], F32)
    b2_sb = sb("b2_sb", [CL2, 1], F32)
    bl_sb = sb("bl_sb", [OUT_F, 1], F32)
    gamma_sb = sb("gamma_sb", [128, 1], F32)
    beta_sb = sb("beta_sb", [128, 1], F32)
    tx_ring = [sb(f"tx{i}", [128, SH], BF16) for i in range(3)]
    ystage = sb("ystage", [128, SH], BF16)
    acc1 = sb("acc1", [128, SH], F32)
    acc2 = sb("acc2", [CL2, SH], F32)
    h_sb = sb("h_sb", [128, SH], BF16)
    stat_sb = sb("stat_sb", [128, 2], F32)
    statg_sb = sb("statg_sb", [128, 2], F32)
    tmp1 = sb("tmp1", [128, 1], F32)
    tmp2 = sb("tmp2", [128, 1], F32)
    abn_sb = sb("abn_sb", [128, 1], F32)
    cbn_sb = sb("cbn_sb", [128, 1], F32)

    RG = [list(range(NCORE))]

    with tile.TileContext(nc) as tc:
        nc.gpsimd.load_library(library_config.mlp)
        with (
            tc.tile_pool(name="g", bufs=GBUFS) as gp,
            tc.tile_pool(name="m", bufs=MBUFS) as mp,
            tc.tile_pool(name="io", bufs=2) as iop,
            tc.tile_pool(name="pa", bufs=PABUFS, space="PSUM") as pa,
            tc.tile_pool(name="pt", bufs=PTBUFS, space="PSUM") as pt,
            tc.tile_pool(name="pg", bufs=2, space="PSUM") as pg,
        ):
            # ---- load persistent inputs
            for dst_, src_ in (
                (idx_lo_sb, idx_lo), (idx_hi_sb, idx_hi),
                (ldst_lo_sb, ldst_lo), (ldst_hi_sb, ldst_hi),
                (dsl_sb, dsl_in), (dsh_sb, dsh_in),
                (iota_sb, iota_in), (eye_sb, eye_in),
                (w1_sb, w1), (w2_sb, w2), (wl_sb, wl),
                (sc_k0_sb, sc_k0), (sc_k_sb, sc_k), (dinv_sh_sb, dinv_sh),
                (b1_sb, b1), (b2_sb, b2), (bl_sb, bl),
                (gamma_sb, gamma), (beta_sb, beta),
            ):
                nc.sync.dma_start(dst_.ap(), src_.ap())

            # ---- init shard: Tx0 ring + GEMM k=0 term
            nc.vector.memset(tx_ring[0].ap(), 0.0)  # Tx_{-1}
            for b in range(NBLK):
                xt = iop.tile([128, F], F32, tag="xsh")
                nc.sync.dma_start(xt[:], xsh.ap()[b * BLK : (b + 1) * BLK, :])
                t0 = tx_ring[1].ap()[:, b * BLK : (b + 1) * BLK]
                nc.vector.tensor_copy(t0, xt[:])  # Tx0 bf16 node-major
                trp = pt.tile([128, 128], BF16, tag="trp")
                nc.tensor.transpose(trp[:], t0, eye_sb.ap())
                trs = mp.tile([128, 128], BF16, tag="trs")
                nc.scalar.copy(trs[:], trp[:])
                gmp = pg.tile([128, 128], F32, tag="gmp")
                nc.tensor.matmul(
                    gmp[:], w1_sb.ap()[:, 0:CL1], trs[:], start=True, stop=True
                )
                nc.vector.tensor_copy(
                    acc1.ap()[:, b * BLK : (b + 1) * BLK], gmp[:]
                )

            # ---- one cheb layer = 3 gather rounds
            def round_(r, layer, kk, tbl_src, ring_prev2, ring_out, w_sb, acc,
                       clo, agin):
                """r: global round id (1..5 for tables), kk: cheb k being
                produced (1..3), clo: out channels."""
                sc_sb = sc_k0_sb if kk == 1 else sc_k_sb
                # gathers for both streams
                gbuf = {}
                for p, (tot_t, idx_sbuf) in enumerate(
                    ((TLO, idx_lo_sb), (THI, idx_hi_sb))
                ):
                    tiles = []
                    for ci, t0 in enumerate(range(0, tot_t, GCH)):
                        n = min(GCH, tot_t - t0)
                        gt = gp.tile([128, GCH, F], BF16, tag=f"g{p}")
                        base = 0 if p == 0 else LO_LIM
                        hi_end = NPAD if p == 1 else LO_LIM
                        if not NOGATHER:
                            nc.gpsimd.dma_gather(
                                gt[:, :n, :],
                                tbl_src.ap()[base:hi_end, :],
                                idx_sbuf.ap()[:, t0 * 8 : (t0 + n) * 8],
                                n * BLK,
                                n * BLK,
                                F,
                                queue_num=0,
                            )
                        tiles.append((t0, n, gt))
                    gbuf[p] = tiles

                # M tiles generated in batches; for the x-table round the
                # one-hot value is dinv[src] instead of 1.0
                wsrc = r == 1
                def m_batches(tot_t, ldst_sbuf, ds_sbuf, p):
                    out = {}
                    for t0 in range(0, tot_t, MB):
                        n = min(MB, tot_t - t0)
                        mt = mp.tile([128, MB, 128], BF16, tag=f"m{p}")
                        if not NOMGEN:
                            nc.vector.tensor_tensor(
                                mt[:, :n, :],
                                iota_sb.ap().unsqueeze(1).broadcast_to(
                                    [128, n, 128]
                                ),
                                ldst_sbuf.ap()[:, t0 : t0 + n]
                                .unsqueeze(2)
                                .broadcast_to([128, n, 128]),
                                OP.is_equal,
                            )
                            if wsrc:
                                nc.vector.tensor_tensor(
                                    mt[:, :n, :],
                                    mt[:, :n, :],
                                    ds_sbuf.ap()[:, t0 : t0 + n]
                                    .unsqueeze(2)
                                    .broadcast_to([128, n, 128]),
                                    OP.mult,
                                )
                        out[t0] = mt
                    return out

                mlo = m_batches(TLO, ldst_lo_sb, dsl_sb, 0)
                mhi = m_batches(THI, ldst_hi_sb, dsh_sb, 1)

                def g_at(p, t):
                    for t0, n, gt in gbuf[p]:
                        if t0 <= t < t0 + n:
                            return gt[:, t - t0, :]
                    raise AssertionError

                def m_at(md, t):
                    t0 = (t // MB) * MB
                    return md[t0][:, t - t0, :]

                off = [0, 0]
                for b in range(NBLK):
                    ps = pa.tile([128, F], F32, tag="agg")
                    ntl, nth = int(T[0][b]), int(T[1][b])
                    first = True
                    for p, nt, md in ((0, ntl, mlo), (1, nth, mhi)):
                        for t in range(nt):
                            tt = off[p] + t
                            if not NOMM:
                                nc.tensor.matmul(
                                    ps[:],
                                    m_at(md, tt),
                                    g_at(p, tt),
                                    start=first,
                                    stop=(p == 1 and t == nth - 1),
                                )
                            first = False
                    off[0] += ntl
                    off[1] += nth

                    blk = slice(b * BLK, (b + 1) * BLK)
                    if not NOEPI:
                        # Tx_next = sc*u - Tx_prev2
                        nc.vector.scalar_tensor_tensor(
                            ring_out.ap()[:, blk],
                            ps[:],
                            sc_sb.ap()[:, b : b + 1],
                            ring_prev2.ap()[:, blk],
                            op0=OP.mult,
                            op1=OP.subtract,
                        )
                        # table row staging: Y = dinv * Tx_next
                        if agin is not None:
                            if YSACT:
                                nc.scalar.mul(
                                    ystage.ap()[:, blk],
                                    ring_out.ap()[:, blk],
                                    dinv_sh_sb.ap()[:, b : b + 1],
                                )
                            else:
                                nc.vector.tensor_scalar(
                                    ystage.ap()[:, blk],
                                    ring_out.ap()[:, blk],
                                    dinv_sh_sb.ap()[:, b : b + 1],
                                    None,
                                    op0=OP.mult,
                                )
                    if not (NOEPI or NOGEMM):
                        # GEMM term k=kk
                        trp = pt.tile([128, 128], BF16, tag="trp")
                        nc.tensor.transpose(
                            trp[:], ring_out.ap()[:, blk], eye_sb.ap()
                        )
                        trs = mp.tile([128, 128], BF16, tag="trs")
                        if _env("TRSDVE", "0"):
                            nc.vector.tensor_copy(trs[:], trp[:])
                        else:
                            nc.scalar.copy(trs[:], trp[:])
                        gmp = pg.tile([clo, 128], F32, tag="gmp")
                        nc.tensor.matmul(
                            gmp[:],
                            w_sb.ap()[:, kk * clo : (kk + 1) * clo],
                            trs[:],
                            start=True,
                            stop=True,
                        )
                        a_blk = (
                            acc.ap()[:clo, blk] if clo < 128 else acc.ap()[:, blk]
                        )
                        nc.vector.tensor_tensor(a_blk, a_blk, gmp[:], OP.add)

                if agin is not None and not NOAG:
                    nc.sync.dma_start(
                        agin.ap().rearrange("(b p) f -> p b f", p=BLK),
                        ystage.ap().rearrange("p (b f) -> p b f", f=F),
                    )
                    nc.gpsimd.collective_compute(
                        "AllGather",
                        OP.bypass,
                        replica_groups=RG,
                        ins=[agin.ap()],
                        outs=[tbls[r].ap()],
                    )

            # ---- layer 1 rounds (produce Tx1..Tx3)
            order = [(1, 1, tbls[0]), (2, 2, tbls[1]), (3, 3, tbls[2])][:NR1]
            prev2, prev1 = tx_ring[0], tx_ring[1]
            free = tx_ring[2]
            for (r, kk, tsrc) in order:
                agin = agins[r] if kk < 3 else None
                round_(r, 1, kk, tsrc, prev2, free, w1_sb, acc1, CL1, agin)
                prev2, prev1, free = prev1, free, prev2

            if PH >= 1:
                # ---- BN + relu
                nc.scalar.activation(
                    acc1.ap(), acc1.ap(), mybir.ActivationFunctionType.Relu,
                    bias=b1_sb.ap(), scale=1.0,
                )
                nc.vector.tensor_reduce(
                    stat_sb.ap()[:, 0:1], acc1.ap(), axis=mybir.AxisListType.X,
                    op=OP.add,
                )
                nc.vector.tensor_tensor(
                    ystage.ap(), acc1.ap(), acc1.ap(), OP.mult
                )
                nc.vector.tensor_reduce(
                    stat_sb.ap()[:, 1:2], ystage.ap(),
                    axis=mybir.AxisListType.X, op=OP.add,
                )
                nc.sync.dma_start(bnc_in.ap(), stat_sb.ap())
                nc.gpsimd.collective_compute(
                    "AllReduce", OP.add, replica_groups=RG,
                    ins=[bnc_in.ap()], outs=[bnc_out.ap()],
                )
                nc.sync.dma_start(statg_sb.ap(), bnc_out.ap())
                # pad-column correction: NPAD-N cols of relu(b1) were summed
                nc.scalar.activation(
                    tmp1.ap(), b1_sb.ap(), mybir.ActivationFunctionType.Relu,
                )
                PADN = float(NPAD - N)
                nc.vector.scalar_tensor_tensor(
                    statg_sb.ap()[:, 0:1], tmp1.ap(), -PADN,
                    statg_sb.ap()[:, 0:1], op0=OP.mult, op1=OP.add,
                )
                nc.vector.tensor_tensor(tmp2.ap(), tmp1.ap(), tmp1.ap(), OP.mult)
                nc.vector.scalar_tensor_tensor(
                    statg_sb.ap()[:, 1:2], tmp2.ap(), -PADN,
                    statg_sb.ap()[:, 1:2], op0=OP.mult, op1=OP.add,
                )
                # mu = s1/N ; var = s2/N - mu^2 ; a = gamma*rsqrt(var+eps)
                mu = tmp1
                nc.vector.tensor_scalar(
                    mu.ap(), statg_sb.ap()[:, 0:1], 1.0 / N, None, op0=OP.mult
                )
                var = tmp2
                nc.vector.tensor_tensor(var.ap(), mu.ap(), mu.ap(), OP.mult)
                nc.vector.scalar_tensor_tensor(
                    var.ap(), statg_sb.ap()[:, 1:2], 1.0 / N, var.ap(),
                    op0=OP.mult, op1=OP.subtract,
                )
                nc.vector.tensor_scalar(
                    var.ap(), var.ap(), float(BN_EPS), None, op0=OP.add
                )
                nc.scalar.activation(
                    var.ap(), var.ap(), mybir.ActivationFunctionType.Sqrt,
                )
                nc.vector.reciprocal(var.ap(), var.ap())
                nc.vector.tensor_tensor(abn_sb.ap(), gamma_sb.ap(), var.ap(),
                                        OP.mult)
                nc.vector.scalar_tensor_tensor(
                    cbn_sb.ap(), mu.ap(), -1.0, abn_sb.ap(),
                    op0=OP.mult, op1=OP.mult,
                )
                nc.vector.tensor_tensor(cbn_sb.ap(), cbn_sb.ap(), beta_sb.ap(),
                                        OP.add)
                # h = a*z + c  (f-major bf16)
                nc.vector.tensor_scalar(
                    h_sb.ap(), acc1.ap(), abn_sb.ap(), cbn_sb.ap(),
                    op0=OP.mult, op1=OP.add,
                )

            if PH >= 2:
                # ---- layer 2 init: ring Tx0' (node-major), table h'=dinv*h, GEMM
                nc.vector.memset(tx_ring[0].ap(), 0.0)
                for b in range(NBLK):
                    blk = slice(b * BLK, (b + 1) * BLK)
                    trp = pt.tile([128, 128], BF16, tag="trp")
                    nc.tensor.transpose(trp[:], h_sb.ap()[:, blk], eye_sb.ap())
                    t0 = tx_ring[1].ap()[:, blk]
                    nc.scalar.copy(t0, trp[:])
                    nc.vector.tensor_scalar(
                        ystage.ap()[:, blk], t0, dinv_sh_sb.ap()[:, b : b + 1],
                        None, op0=OP.mult,
                    )
                    gmp = pg.tile([CL2, 128], F32, tag="gmp")
                    nc.tensor.matmul(
                        gmp[:], w2_sb.ap()[:, 0:CL2], h_sb.ap()[:, blk],
                        start=True, stop=True,
                    )
                    nc.vector.tensor_copy(acc2.ap()[:, blk], gmp[:])
                if not NOAG:
                    nc.sync.dma_start(
                        agins[3].ap().rearrange("(b p) f -> p b f", p=BLK),
                        ystage.ap().rearrange("p (b f) -> p b f", f=F),
                    )
                    nc.gpsimd.collective_compute(
                        "AllGather", OP.bypass, replica_groups=RG,
                        ins=[agins[3].ap()], outs=[tbls[3].ap()],
                    )

            if PH >= 3:
                # ---- layer 2 rounds
                prev2, prev1, free = tx_ring[0], tx_ring[1], tx_ring[2]
                order = [(4, 1, tbls[3]), (5, 2, tbls[4]), (6, 3, tbls[5])][:NR2]
                for (r, kk, tsrc) in order:
                    agin = agins[r] if kk < 3 else None
                    round_(r, 2, kk, tsrc, prev2, free, w2_sb, acc2, CL2, agin)
                    prev2, prev1, free = prev1, free, prev2

            if PH >= 4:
                # ---- head
                nc.scalar.activation(
                    acc2.ap(), acc2.ap(), mybir.ActivationFunctionType.Relu,
                    bias=b2_sb.ap(), scale=1.0,
                )
                for b in range(NBLK):
                    blk = slice(b * BLK, (b + 1) * BLK)
                    hp = pg.tile([OUT_F, 128], F32, tag="hd")
                    nc.tensor.matmul(
                        hp[:], wl_sb.ap(), acc2.ap()[:, blk], start=True,
                        stop=True,
                    )
                    nc.scalar.activation(
                        acc1.ap()[0:OUT_F, blk], hp[:],
                        mybir.ActivationFunctionType.Identity, bias=bl_sb.ap(),
                    )
            nc.sync.dma_start(out_t.ap(), acc1.ap()[0:OUT_F, :])

    nc.compile()
    return nc




def _make_inmaps(inputs, dinv, streams):
    bf = ml_dtypes.bfloat16
    x = np.asarray(inputs["x"], np.float32)
    xp = np.zeros((NPAD, F), np.float32)
    xp[:N] = x
    xbf = xp.astype(bf)
    W1 = np.asarray(inputs["W1"], np.float32)
    W2 = np.asarray(inputs["W2"], np.float32)
    iota_np = np.tile(
        np.arange(128, dtype=np.float32)[None, :], (128, 1)
    ).astype(bf)
    eye_np = np.eye(128, dtype=np.float32).astype(bf)
    w1_np = np.ascontiguousarray(
        np.transpose(W1, (1, 0, 2)).reshape(F, K * CL1)
    ).astype(bf)
    w2_np = np.ascontiguousarray(
        np.transpose(W2, (1, 0, 2)).reshape(CL1, K * CL2)
    ).astype(bf)
    in_maps = []
    for c in range(NCORE):
        sl = slice(c * SH, (c + 1) * SH)
        dsh = dinv[sl].reshape(NBLK, BLK).T.copy()
        ilo, dlo = streams[c][0]
        ihi, dhi = streams[c][1]
        dsl_v = np.where(dlo >= 0, dinv[ilo.astype(np.int64)], 0.0)
        dsh_v = np.where(
            dhi >= 0, dinv[ihi.astype(np.int64) + LO_LIM], 0.0
        )
        in_maps.append(
            {
                "xbf": xbf,
                "dsl": _ldst_tiles_f(dsl_v),
                "dsh": _ldst_tiles_f(dsh_v),
                "xsh": xp[sl].copy(),
                "idx_lo": _wrap_idx16(ilo),
                "idx_hi": _wrap_idx16(ihi),
                "ldst_lo": _ldst_tiles(dlo),
                "ldst_hi": _ldst_tiles(dhi),
                "sc_k0": -dsh,
                "sc_k": -2.0 * dsh,
                "dinv_sh": dsh,
                "w1": w1_np,
                "w2": w2_np,
                "wl": np.asarray(inputs["Wl"], np.float32),
                "b1": np.asarray(inputs["b1"], np.float32).reshape(128, 1),
                "b2": np.asarray(inputs["b2"], np.float32).reshape(CL2, 1),
                "bl": np.asarray(inputs["bl"], np.float32).reshape(OUT_F, 1),
                "gamma": np.asarray(inputs["gamma"], np.float32).reshape(
                    128, 1
                ),
                "beta": np.asarray(inputs["beta"], np.float32).reshape(
                    128, 1
                ),
                "iota": iota_np,
                "eye": eye_np,
            }
        )
    return in_maps

# ------------------------------------------------------------------- driver
_CACHE = {}


def kernel(x, edge_index, W1, b1, gamma, beta, W2, b2, Wl, bl):
    x = np.asarray(x, np.float32)
    edge_index = np.asarray(edge_index)
    W1 = np.asarray(W1, np.float32)
    W2 = np.asarray(W2, np.float32)

    dinv, T, streams = _preprocess(edge_index)

    NR1 = _env("NR1", "3")
    NR2 = _env("NR2", "3")
    PH = _env("PH", "9")
    key = (T.tobytes(), NR1, NR2, PH, GCH, MB, GBUFS, MBUFS, PABUFS, PTBUFS, YSACT, NOAG, NOMM, NOGATHER, NOMGEN, NOEPI, NOGEMM)
    if key not in _CACHE:
        _CACHE[key] = _build(T, NR1, NR2, PH)
    nc = _CACHE[key]

    in_maps = _make_inmaps(
        dict(x=x, W1=W1, b1=b1, gamma=gamma, beta=beta, W2=W2, b2=b2,
             Wl=Wl, bl=bl), dinv, streams)

    res = run_bass_kernel_spmd(nc, in_maps, core_ids=list(range(NCORE)))
    kernel.last_results = res

    out = np.empty((N, OUT_F), np.float32)
    for c in range(NCORE):
        lo = c * SH
        hi = min((c + 1) * SH, N)
        out[lo:hi] = res.results[c]["out"].T[: hi - lo]
    return out


# ------------------------------------------------------------------ timing
def bench(iters=8, **inputs):
    """Wall-clock the NEFF execution via the PJRT path (min over iters).

    Includes PJRT/axon dispatch overhead; use as an upper bound on HW
    exec time and for relative optimization."""
    import time
    import jax
    from jax.sharding import Mesh, PartitionSpec, NamedSharding
    from jax.experimental.shard_map import shard_map
    from concourse import bass2jax
    import concourse.mybir as mybir_

    x = np.asarray(inputs["x"], np.float32)
    edge_index = np.asarray(inputs["edge_index"])
    dinv, T, streams = _preprocess(edge_index)
    NR1 = _env("NR1", "3")
    NR2 = _env("NR2", "3")
    PH = _env("PH", "9")
    key = (T.tobytes(), NR1, NR2, PH, GCH, MB, GBUFS, MBUFS, PABUFS, PTBUFS, YSACT, NOAG, NOMM, NOGATHER, NOMGEN, NOEPI, NOGEMM)
    if key not in _CACHE:
        _CACHE[key] = _build(T, NR1, NR2, PH)
    nc = _CACHE[key]
    in_maps = _make_inmaps(inputs, dinv, streams)

    bass2jax.install_neuronx_cc_hook()
    partition_name = (
        nc.partition_id_tensor.name if nc.partition_id_tensor else None
    )
    in_names, out_names, out_avals, zero_outs = [], [], [], []
    for alloc in nc.m.functions[0].allocations:
        if not isinstance(alloc, mybir_.MemoryLocationSet):
            continue
        name = alloc.memorylocations[0].name
        if alloc.kind == "ExternalInput":
            if name != partition_name:
                in_names.append(name)
        elif alloc.kind == "ExternalOutput":
            shape = tuple(alloc.tensor_shape)
            dtype = mybir_.dt.np(alloc.dtype)
            out_avals.append(jax.core.ShapedArray(shape, dtype))
            zero_outs.append(np.zeros(shape, dtype))
            out_names.append(name)
    n_params = len(in_names)
    n_outs = len(out_avals)
    in_names.extend(out_names)
    if partition_name is not None:
        in_names.append(partition_name)
    donate = tuple(range(n_params, n_params + n_outs))

    def _body(*args):
        operands = list(args)
        if partition_name is not None:
            operands.append(bass2jax.partition_id_tensor())
        outs = bass2jax._bass_exec_p.bind(
            *operands,
            out_avals=tuple(out_avals),
            in_names=tuple(in_names),
            out_names=tuple(out_names),
            lowering_input_output_aliases=(),
            sim_require_finite=True,
            sim_require_nnan=True,
            nc=nc,
        )
        return tuple(outs)

    devices = jax.devices()[:NCORE]
    mesh = Mesh(np.asarray(devices), ("core",))
    in_specs = (PartitionSpec("core"),) * (n_params + n_outs)
    out_specs = (PartitionSpec("core"),) * len(out_names)
    sharded = jax.jit(
        shard_map(
            _body, mesh=mesh, in_specs=in_specs, out_specs=out_specs,
            check_rep=False,
        ),
        donate_argnums=donate,
        keep_unused=True,
    )
    per_core = [
        [np.asarray(m[name]) for name in in_names[:n_params]] for m in in_maps
    ]
    sh = NamedSharding(mesh, PartitionSpec("core"))
    concat_in = [
        jax.device_put(
            np.concatenate([per_core[c][i] for c in range(NCORE)], axis=0), sh
        )
        for i in range(n_params)
    ]

    times = []
    for it in range(iters):
        concat_zeros = [
            np.zeros((NCORE * z.shape[0], *z.shape[1:]), z.dtype)
            for z in zero_outs
        ]
        t0 = time.perf_counter()
        out_arrs = sharded(*concat_in, *concat_zeros)
        jax.block_until_ready(out_arrs)
        times.append(time.perf_counter() - t0)
    return times



# revision 9
# speedup vs baseline: 16.4067x; 16.4067x over previous
"""Distributed Trainium2 Bass kernel for a 2-layer ChebConv GCN (K=4).

Strategy (8 NeuronCores, node sharding):
  - Nodes padded to 50176 = 8 shards x 6272 (49 blocks of 128).
  - The gather table Y_k = dinv * Tx_k (node-major bf16 rows, 256B) is
    replicated in every core's HBM; per Chebyshev hop each core
    dma_gather's the source rows of its destination-sharded edges,
    reduces them per 128-dst block with one-hot matmuls on the
    TensorEngine (one-hot built on the VectorEngine from edge metadata),
    and applies the recurrence Tx_{k+1} = sc*u - Tx_{k-1} on PSUM exit.
  - Halo exchange = AllGather of each core's recomputed shard.
  - Edges are bucketed by (dst shard, dst block) on the host and split
    lo/hi on src < 32768 (int16 gather index limit); tile counts are
    equalized across cores so all 8 cores run one SPMD program.
  - Dense GEMMs (x@W per hop), BN (stats AllReduce), ReLU, and the
    linear head run on-device, feature-major, fused into the rounds.
"""
import sys

sys.path.insert(0, "/opt/trn_rl_repo")

import numpy as np
import ml_dtypes

import concourse.bass as bass
import concourse.bacc as bacc
import concourse.tile as tile
import concourse.mybir as mybir
from concourse import library_config
from concourse.bass_utils import run_bass_kernel_spmd

BF16 = mybir.dt.bfloat16
F32 = mybir.dt.float32
I16 = mybir.dt.int16
OP = mybir.AluOpType

N = 50000
E = 800000
NCORE = 8
BLK = 128
NBLK = 49                 # blocks per shard
SH = NBLK * BLK           # 6272
NPAD = NCORE * SH         # 50176
NBG = NPAD // BLK         # 392 global blocks
LO_LIM = 32768
F = 128                   # feature width of both cheb layers' inputs
CL1 = 128
CL2 = 64
OUT_F = 10
K = 4
BN_EPS = 1e-5

import os as _os


def _env(name, default):
    return int(_os.environ.get("CHEBK_" + name, default))


GCH = _env("GCH", "8")     # gather chunk, tiles of 128 edges
GQS = _env("GQS", "4")     # spread gathers across this many SWDGE queues
NOAG = _env("NOAG", "0")
NOMM = _env("NOMM", "0")
NOGATHER = _env("NOGATHER", "0")
NOMGEN = _env("NOMGEN", "0")
NOEPI = _env("NOEPI", "0")
NOGEMM = _env("NOGEMM", "0")
MB = _env("MB", "8")       # M-gen batch, tiles per DVE instr
GBUFS = _env("GBUFS", "6")
MBUFS = _env("MBUFS", "4")
PABUFS = _env("PABUFS", "3")
PTBUFS = _env("PTBUFS", "1")
YSACT = _env("YSACT", "1")


# ----------------------------------------------------------------- host prep
def _preprocess(edge_index):
    src = np.asarray(edge_index[0], dtype=np.int64)
    dst = np.asarray(edge_index[1], dtype=np.int64)

    deg = np.bincount(src, minlength=NPAD).astype(np.float64)
    dinv = np.zeros(NPAD, np.float32)
    nz = deg > 0
    dinv[nz] = (1.0 / np.sqrt(deg[nz])).astype(np.float32)

    shard = dst // SH
    block = (dst % SH) // BLK
    ld = (dst % BLK).astype(np.int16)
    is_hi = (src >= LO_LIM).astype(np.int64)

    # order edges by (core, part, block)
    key = ((shard * 2 + is_hi) * NBLK + block).astype(np.int64)
    order = np.argsort(key, kind="stable")
    counts = np.bincount(key, minlength=NCORE * 2 * NBLK).reshape(
        NCORE, 2, NBLK
    )

    T = np.maximum(1, -(-counts.max(axis=0) // BLK))  # [2, NBLK]

    # per-core slot streams
    starts = np.zeros(NCORE * 2 * NBLK + 1, np.int64)
    np.cumsum(counts.reshape(-1), out=starts[1:])
    idx_s, ld_s = src[order], ld[order]

    streams = []
    tot = {0: int(T[0].sum()) * BLK, 1: int(T[1].sum()) * BLK}
    for c in range(NCORE):
        out = {}
        for p in range(2):
            ii = np.zeros(tot[p], np.int16)
            dd = np.full(tot[p], -1, np.int16)
            off = 0
            for b in range(NBLK):
                k = (c * 2 + p) * NBLK + b
                s0, s1 = starts[k], starts[k + 1]
                n = s1 - s0
                base = 0 if p == 0 else LO_LIM
                ii[off : off + n] = (idx_s[s0:s1] - base).astype(np.int16)
                dd[off : off + n] = ld_s[s0:s1]
                off += T[p, b] * BLK
            out[p] = (ii, dd)
        streams.append(out)
    return dinv, T, streams


def _wrap_idx16(a):
    return np.tile(a.reshape(-1, 16).T, (8, 1)).copy()


def _ldst_tiles(a):
    nt = a.size // BLK
    return a.reshape(nt, BLK).T.astype(np.float32).copy()


def _ldst_tiles_f(a):
    nt = a.size // BLK
    return np.ascontiguousarray(
        a.reshape(nt, BLK).T.astype(np.float32)
    )


# ------------------------------------------------------------- program build
def _build(T, NR1=3, NR2=3, PH=9):
    TLO, THI = int(T[0].sum()), int(T[1].sum())
    SLO, SHI = TLO * BLK, THI * BLK

    nc = bacc.Bacc(
        "TRN2", target_bir_lowering=False, debug=False, num_devices=NCORE,
        num_swdge_queues=max(1, min(4, GQS)),
    )

    def din(name, shape, dt):
        return nc.dram_tensor(name, shape, dt, kind="ExternalInput")

    xbf = din("xbf", [NPAD, F], BF16)
    xsh = din("xsh", [SH, F], F32)
    idx_lo = din("idx_lo", [128, SLO // 16], I16)
    idx_hi = din("idx_hi", [128, SHI // 16], I16)
    ldst_lo = din("ldst_lo", [128, TLO], F32)
    ldst_hi = din("ldst_hi", [128, THI], F32)
    dsl_in = din("dsl", [128, TLO], F32)
    dsh_in = din("dsh", [128, THI], F32)
    sc_k0 = din("sc_k0", [128, NBLK], F32)   # -dinv (shard)
    sc_k = din("sc_k", [128, NBLK], F32)     # -2*dinv (shard)
    dinv_sh = din("dinv_sh", [128, NBLK], F32)
    w1 = din("w1", [128, K * CL1], BF16)
    w2 = din("w2", [128, K * CL2], BF16)
    wl = din("wl", [CL2, OUT_F], F32)
    b1 = din("b1", [128, 1], F32)
    b2 = din("b2", [CL2, 1], F32)
    bl = din("bl", [OUT_F, 1], F32)
    gamma = din("gamma", [128, 1], F32)
    beta = din("beta", [128, 1], F32)
    iota_in = din("iota", [128, 128], BF16)
    eye_in = din("eye", [128, 128], BF16)

    out_t = nc.dram_tensor("out", [OUT_F, SH], F32, kind="ExternalOutput")

    # internal dram
    tbls = [xbf] + [
        nc.dram_tensor(f"tbl{r}", [NPAD, F], BF16, addr_space="Shared")
        for r in range(1, 6)
    ]
    agins = {
        r: nc.dram_tensor(f"agin{r}", [SH, F], BF16) for r in (1, 2, 3, 4, 5)
    }
    bnc_in = nc.dram_tensor("bnc_in", [128, 2], F32)
    bnc_out = nc.dram_tensor("bnc_out", [128, 2], F32, addr_space="Shared")

    # persistent SBUF
    sb = lambda n, s, d: nc.alloc_sbuf_tensor(n, s, d)
    idx_lo_sb = sb("idx_lo_sb", [128, SLO // 16], I16)
    idx_hi_sb = sb("idx_hi_sb", [128, SHI // 16], I16)
    ldst_lo_sb = sb("ldst_lo_sb", [128, TLO], F32)
    ldst_hi_sb = sb("ldst_hi_sb", [128, THI], F32)
    dsl_sb = sb("dsl_sb", [128, TLO], F32)
    dsh_sb = sb("dsh_sb", [128, THI], F32)
    iota_sb = sb("iota_sb", [128, 128], BF16)
    eye_sb = sb("eye_sb", [128, 128], BF16)
    w1_sb = sb("w1_sb", [128, K * CL1], BF16)
    w2_sb = sb("w2_sb", [128, K * CL2], BF16)
    wl_sb = sb("wl_sb", [CL2, OUT_F], F32)
    sc_k0_sb = sb("sc_k0_sb", [128, NBLK], F32)
    sc_k_sb = sb("sc_k_sb", [128, NBLK], F32)
    dinv_sh_sb = sb("dinv_sh_sb", [128, NBLK], F32)
    b1_sb = sb("b1_sb", [128, 1], F32)
    b2_sb = sb("b2_sb", [CL2, 1], F32)
    bl_sb = sb("bl_sb", [OUT_F, 1], F32)
    gamma_sb = sb("gamma_sb", [128, 1], F32)
    beta_sb = sb("beta_sb", [128, 1], F32)
    tx_ring = [sb(f"tx{i}", [128, SH], BF16) for i in range(3)]
    ystage = sb("ystage", [128, SH], BF16)
    acc1 = sb("acc1", [128, SH], F32)
    acc2 = sb("acc2", [CL2, SH], F32)
    h_sb = sb("h_sb", [128, SH], BF16)
    stat_sb = sb("stat_sb", [128, 2], F32)
    statg_sb = sb("statg_sb", [128, 2], F32)
    tmp1 = sb("tmp1", [128, 1], F32)
    tmp2 = sb("tmp2", [128, 1], F32)
    abn_sb = sb("abn_sb", [128, 1], F32)
    cbn_sb = sb("cbn_sb", [128, 1], F32)

    RG = [list(range(NCORE))]

    with tile.TileContext(nc) as tc:
        nc.gpsimd.load_library(library_config.mlp)
        with (
            tc.tile_pool(name="g", bufs=GBUFS) as gp,
            tc.tile_pool(name="m", bufs=MBUFS) as mp,
            tc.tile_pool(name="io", bufs=2) as iop,
            tc.tile_pool(name="pa", bufs=PABUFS, space="PSUM") as pa,
            tc.tile_pool(name="pt", bufs=PTBUFS, space="PSUM") as pt,
            tc.tile_pool(name="pg", bufs=2, space="PSUM") as pg,
        ):
            # ---- load persistent inputs
            for dst_, src_ in (
                (idx_lo_sb, idx_lo), (idx_hi_sb, idx_hi),
                (ldst_lo_sb, ldst_lo), (ldst_hi_sb, ldst_hi),
                (dsl_sb, dsl_in), (dsh_sb, dsh_in),
                (iota_sb, iota_in), (eye_sb, eye_in),
                (w1_sb, w1), (w2_sb, w2), (wl_sb, wl),
                (sc_k0_sb, sc_k0), (sc_k_sb, sc_k), (dinv_sh_sb, dinv_sh),
                (b1_sb, b1), (b2_sb, b2), (bl_sb, bl),
                (gamma_sb, gamma), (beta_sb, beta),
            ):
                nc.sync.dma_start(dst_.ap(), src_.ap())

            # ---- init shard: Tx0 ring + GEMM k=0 term
            nc.vector.memset(tx_ring[0].ap(), 0.0)  # Tx_{-1}
            for b in range(NBLK):
                xt = iop.tile([128, F], F32, tag="xsh")
                nc.sync.dma_start(xt[:], xsh.ap()[b * BLK : (b + 1) * BLK, :])
                t0 = tx_ring[1].ap()[:, b * BLK : (b + 1) * BLK]
                nc.vector.tensor_copy(t0, xt[:])  # Tx0 bf16 node-major
                trp = pt.tile([128, 128], BF16, tag="trp")
                nc.tensor.transpose(trp[:], t0, eye_sb.ap())
                trs = mp.tile([128, 128], BF16, tag="trs")
                nc.scalar.copy(trs[:], trp[:])
                gmp = pg.tile([128, 128], F32, tag="gmp")
                nc.tensor.matmul(
                    gmp[:], w1_sb.ap()[:, 0:CL1], trs[:], start=True, stop=True
                )
                nc.vector.tensor_copy(
                    acc1.ap()[:, b * BLK : (b + 1) * BLK], gmp[:]
                )

            # ---- one cheb layer = 3 gather rounds
            def round_(r, layer, kk, tbl_src, ring_prev2, ring_out, w_sb, acc,
                       clo, agin):
                """r: global round id (1..5 for tables), kk: cheb k being
                produced (1..3), clo: out channels."""
                sc_sb = sc_k0_sb if kk == 1 else sc_k_sb
                # gathers for both streams
                gbuf = {}
                for p, (tot_t, idx_sbuf) in enumerate(
                    ((TLO, idx_lo_sb), (THI, idx_hi_sb))
                ):
                    tiles = []
                    for ci, t0 in enumerate(range(0, tot_t, GCH)):
                        n = min(GCH, tot_t - t0)
                        gt = gp.tile([128, GCH, F], BF16, tag=f"g{p}")
                        base = 0 if p == 0 else LO_LIM
                        hi_end = NPAD if p == 1 else LO_LIM
                        if not NOGATHER:
                            nc.gpsimd.dma_gather(
                                gt[:, :n, :],
                                tbl_src.ap()[base:hi_end, :],
                                idx_sbuf.ap()[:, t0 * 8 : (t0 + n) * 8],
                                n * BLK,
                                n * BLK,
                                F,
                                queue_num=(ci + p) % GQS,
                            )
                        tiles.append((t0, n, gt))
                    gbuf[p] = tiles

                # M tiles generated in batches; for the x-table round the
                # one-hot value is dinv[src] instead of 1.0
                wsrc = r == 1
                def m_batches(tot_t, ldst_sbuf, ds_sbuf, p):
                    out = {}
                    for t0 in range(0, tot_t, MB):
                        n = min(MB, tot_t - t0)
                        mt = mp.tile([128, MB, 128], BF16, tag=f"m{p}")
                        if not NOMGEN:
                            nc.vector.tensor_tensor(
                                mt[:, :n, :],
                                iota_sb.ap().unsqueeze(1).broadcast_to(
                                    [128, n, 128]
                                ),
                                ldst_sbuf.ap()[:, t0 : t0 + n]
                                .unsqueeze(2)
                                .broadcast_to([128, n, 128]),
                                OP.is_equal,
                            )
                            if wsrc:
                                nc.vector.tensor_tensor(
                                    mt[:, :n, :],
                                    mt[:, :n, :],
                                    ds_sbuf.ap()[:, t0 : t0 + n]
                                    .unsqueeze(2)
                                    .broadcast_to([128, n, 128]),
                                    OP.mult,
                                )
                        out[t0] = mt
                    return out

                mlo = m_batches(TLO, ldst_lo_sb, dsl_sb, 0)
                mhi = m_batches(THI, ldst_hi_sb, dsh_sb, 1)

                def g_at(p, t):
                    for t0, n, gt in gbuf[p]:
                        if t0 <= t < t0 + n:
                            return gt[:, t - t0, :]
                    raise AssertionError

                def m_at(md, t):
                    t0 = (t // MB) * MB
                    return md[t0][:, t - t0, :]

                off = [0, 0]
                for b in range(NBLK):
                    ps = pa.tile([128, F], F32, tag="agg")
                    ntl, nth = int(T[0][b]), int(T[1][b])
                    first = True
                    for p, nt, md in ((0, ntl, mlo), (1, nth, mhi)):
                        for t in range(nt):
                            tt = off[p] + t
                            if not NOMM:
                                nc.tensor.matmul(
                                    ps[:],
                                    m_at(md, tt),
                                    g_at(p, tt),
                                    start=first,
                                    stop=(p == 1 and t == nth - 1),
                                )
                            first = False
                    off[0] += ntl
                    off[1] += nth

                    blk = slice(b * BLK, (b + 1) * BLK)
                    if not NOEPI:
                        # Tx_next = sc*u - Tx_prev2
                        nc.vector.scalar_tensor_tensor(
                            ring_out.ap()[:, blk],
                            ps[:],
                            sc_sb.ap()[:, b : b + 1],
                            ring_prev2.ap()[:, blk],
                            op0=OP.mult,
                            op1=OP.subtract,
                        )
                        # table row staging: Y = dinv * Tx_next
                        if agin is not None:
                            if YSACT:
                                nc.scalar.mul(
                                    ystage.ap()[:, blk],
                                    ring_out.ap()[:, blk],
                                    dinv_sh_sb.ap()[:, b : b + 1],
                                )
                            else:
                                nc.vector.tensor_scalar(
                                    ystage.ap()[:, blk],
                                    ring_out.ap()[:, blk],
                                    dinv_sh_sb.ap()[:, b : b + 1],
                                    None,
                                    op0=OP.mult,
                                )
                    if not (NOEPI or NOGEMM):
                        # GEMM term k=kk
                        trp = pt.tile([128, 128], BF16, tag="trp")
                        nc.tensor.transpose(
                            trp[:], ring_out.ap()[:, blk], eye_sb.ap()
                        )
                        trs = mp.tile([128, 128], BF16, tag="trs")
                        if _env("TRSDVE", "0"):
                            nc.vector.tensor_copy(trs[:], trp[:])
                        else:
                            nc.scalar.copy(trs[:], trp[:])
                        gmp = pg.tile([clo, 128], F32, tag="gmp")
                        nc.tensor.matmul(
                            gmp[:],
                            w_sb.ap()[:, kk * clo : (kk + 1) * clo],
                            trs[:],
                            start=True,
                            stop=True,
                        )
                        a_blk = (
                            acc.ap()[:clo, blk] if clo < 128 else acc.ap()[:, blk]
                        )
                        nc.vector.tensor_tensor(a_blk, a_blk, gmp[:], OP.add)

                if agin is not None and not NOAG:
                    nc.sync.dma_start(
                        agin.ap().rearrange("(b p) f -> p b f", p=BLK),
                        ystage.ap().rearrange("p (b f) -> p b f", f=F),
                    )
                    nc.gpsimd.collective_compute(
                        "AllGather",
                        OP.bypass,
                        replica_groups=RG,
                        ins=[agin.ap()],
                        outs=[tbls[r].ap()],
                    )

            # ---- layer 1 rounds (produce Tx1..Tx3)
            order = [(1, 1, tbls[0]), (2, 2, tbls[1]), (3, 3, tbls[2])][:NR1]
            prev2, prev1 = tx_ring[0], tx_ring[1]
            free = tx_ring[2]
            for (r, kk, tsrc) in order:
                agin = agins[r] if kk < 3 else None
                round_(r, 1, kk, tsrc, prev2, free, w1_sb, acc1, CL1, agin)
                prev2, prev1, free = prev1, free, prev2

            if PH >= 1:
                # ---- BN + relu
                nc.scalar.activation(
                    acc1.ap(), acc1.ap(), mybir.ActivationFunctionType.Relu,
                    bias=b1_sb.ap(), scale=1.0,
                )
                nc.vector.tensor_reduce(
                    stat_sb.ap()[:, 0:1], acc1.ap(), axis=mybir.AxisListType.X,
                    op=OP.add,
                )
                nc.vector.tensor_tensor(
                    ystage.ap(), acc1.ap(), acc1.ap(), OP.mult
                )
                nc.vector.tensor_reduce(
                    stat_sb.ap()[:, 1:2], ystage.ap(),
                    axis=mybir.AxisListType.X, op=OP.add,
                )
                nc.sync.dma_start(bnc_in.ap(), stat_sb.ap())
                nc.gpsimd.collective_compute(
                    "AllReduce", OP.add, replica_groups=RG,
                    ins=[bnc_in.ap()], outs=[bnc_out.ap()],
                )
                nc.sync.dma_start(statg_sb.ap(), bnc_out.ap())
                # pad-column correction: NPAD-N cols of relu(b1) were summed
                nc.scalar.activation(
                    tmp1.ap(), b1_sb.ap(), mybir.ActivationFunctionType.Relu,
                )
                PADN = float(NPAD - N)
                nc.vector.scalar_tensor_tensor(
                    statg_sb.ap()[:, 0:1], tmp1.ap(), -PADN,
                    statg_sb.ap()[:, 0:1], op0=OP.mult, op1=OP.add,
                )
                nc.vector.tensor_tensor(tmp2.ap(), tmp1.ap(), tmp1.ap(), OP.mult)
                nc.vector.scalar_tensor_tensor(
                    statg_sb.ap()[:, 1:2], tmp2.ap(), -PADN,
                    statg_sb.ap()[:, 1:2], op0=OP.mult, op1=OP.add,
                )
                # mu = s1/N ; var = s2/N - mu^2 ; a = gamma*rsqrt(var+eps)
                mu = tmp1
                nc.vector.tensor_scalar(
                    mu.ap(), statg_sb.ap()[:, 0:1], 1.0 / N, None, op0=OP.mult
                )
                var = tmp2
                nc.vector.tensor_tensor(var.ap(), mu.ap(), mu.ap(), OP.mult)
                nc.vector.scalar_tensor_tensor(
                    var.ap(), statg_sb.ap()[:, 1:2], 1.0 / N, var.ap(),
                    op0=OP.mult, op1=OP.subtract,
                )
                nc.vector.tensor_scalar(
                    var.ap(), var.ap(), float(BN_EPS), None, op0=OP.add
                )
                nc.scalar.activation(
                    var.ap(), var.ap(), mybir.ActivationFunctionType.Sqrt,
                )
                nc.vector.reciprocal(var.ap(), var.ap())
                nc.vector.tensor_tensor(abn_sb.ap(), gamma_sb.ap(), var.ap(),
                                        OP.mult)
                nc.vector.scalar_tensor_tensor(
                    cbn_sb.ap(), mu.ap(), -1.0, abn_sb.ap(),
                    op0=OP.mult, op1=OP.mult,
                )
                nc.vector.tensor_tensor(cbn_sb.ap(), cbn_sb.ap(), beta_sb.ap(),
                                        OP.add)
                # h = a*z + c  (f-major bf16)
                nc.vector.tensor_scalar(
                    h_sb.ap(), acc1.ap(), abn_sb.ap(), cbn_sb.ap(),
                    op0=OP.mult, op1=OP.add,
                )

            if PH >= 2:
                # ---- layer 2 init: ring Tx0' (node-major), table h'=dinv*h, GEMM
                nc.vector.memset(tx_ring[0].ap(), 0.0)
                for b in range(NBLK):
                    blk = slice(b * BLK, (b + 1) * BLK)
                    trp = pt.tile([128, 128], BF16, tag="trp")
                    nc.tensor.transpose(trp[:], h_sb.ap()[:, blk], eye_sb.ap())
                    t0 = tx_ring[1].ap()[:, blk]
                    nc.scalar.copy(t0, trp[:])
                    nc.vector.tensor_scalar(
                        ystage.ap()[:, blk], t0, dinv_sh_sb.ap()[:, b : b + 1],
                        None, op0=OP.mult,
                    )
                    gmp = pg.tile([CL2, 128], F32, tag="gmp")
                    nc.tensor.matmul(
                        gmp[:], w2_sb.ap()[:, 0:CL2], h_sb.ap()[:, blk],
                        start=True, stop=True,
                    )
                    nc.vector.tensor_copy(acc2.ap()[:, blk], gmp[:])
                if not NOAG:
                    nc.sync.dma_start(
                        agins[3].ap().rearrange("(b p) f -> p b f", p=BLK),
                        ystage.ap().rearrange("p (b f) -> p b f", f=F),
                    )
                    nc.gpsimd.collective_compute(
                        "AllGather", OP.bypass, replica_groups=RG,
                        ins=[agins[3].ap()], outs=[tbls[3].ap()],
                    )

            if PH >= 3:
                # ---- layer 2 rounds
                prev2, prev1, free = tx_ring[0], tx_ring[1], tx_ring[2]
                order = [(4, 1, tbls[3]), (5, 2, tbls[4]), (6, 3, tbls[5])][:NR2]
                for (r, kk, tsrc) in order:
                    agin = agins[r] if kk < 3 else None
                    round_(r, 2, kk, tsrc, prev2, free, w2_sb, acc2, CL2, agin)
                    prev2, prev1, free = prev1, free, prev2

            if PH >= 4:
                # ---- head
                nc.scalar.activation(
                    acc2.ap(), acc2.ap(), mybir.ActivationFunctionType.Relu,
                    bias=b2_sb.ap(), scale=1.0,
                )
                for b in range(NBLK):
                    blk = slice(b * BLK, (b + 1) * BLK)
                    hp = pg.tile([OUT_F, 128], F32, tag="hd")
                    nc.tensor.matmul(
                        hp[:], wl_sb.ap(), acc2.ap()[:, blk], start=True,
                        stop=True,
                    )
                    nc.scalar.activation(
                        acc1.ap()[0:OUT_F, blk], hp[:],
                        mybir.ActivationFunctionType.Identity, bias=bl_sb.ap(),
                    )
            nc.sync.dma_start(out_t.ap(), acc1.ap()[0:OUT_F, :])

    nc.compile()
    return nc




def _make_inmaps(inputs, dinv, streams):
    bf = ml_dtypes.bfloat16
    x = np.asarray(inputs["x"], np.float32)
    xp = np.zeros((NPAD, F), np.float32)
    xp[:N] = x
    xbf = xp.astype(bf)
    W1 = np.asarray(inputs["W1"], np.float32)
    W2 = np.asarray(inputs["W2"], np.float32)
    iota_np = np.tile(
        np.arange(128, dtype=np.float32)[None, :], (128, 1)
    ).astype(bf)
    eye_np = np.eye(128, dtype=np.float32).astype(bf)
    w1_np = np.ascontiguousarray(
        np.transpose(W1, (1, 0, 2)).reshape(F, K * CL1)
    ).astype(bf)
    w2_np = np.ascontiguousarray(
        np.transpose(W2, (1, 0, 2)).reshape(CL1, K * CL2)
    ).astype(bf)
    in_maps = []
    for c in range(NCORE):
        sl = slice(c * SH, (c + 1) * SH)
        dsh = dinv[sl].reshape(NBLK, BLK).T.copy()
        ilo, dlo = streams[c][0]
        ihi, dhi = streams[c][1]
        dsl_v = np.where(dlo >= 0, dinv[ilo.astype(np.int64)], 0.0)
        dsh_v = np.where(
            dhi >= 0, dinv[ihi.astype(np.int64) + LO_LIM], 0.0
        )
        in_maps.append(
            {
                "xbf": xbf,
                "dsl": _ldst_tiles_f(dsl_v),
                "dsh": _ldst_tiles_f(dsh_v),
                "xsh": xp[sl].copy(),
                "idx_lo": _wrap_idx16(ilo),
                "idx_hi": _wrap_idx16(ihi),
                "ldst_lo": _ldst_tiles(dlo),
                "ldst_hi": _ldst_tiles(dhi),
                "sc_k0": -dsh,
                "sc_k": -2.0 * dsh,
                "dinv_sh": dsh,
                "w1": w1_np,
                "w2": w2_np,
                "wl": np.asarray(inputs["Wl"], np.float32),
                "b1": np.asarray(inputs["b1"], np.float32).reshape(128, 1),
                "b2": np.asarray(inputs["b2"], np.float32).reshape(CL2, 1),
                "bl": np.asarray(inputs["bl"], np.float32).reshape(OUT_F, 1),
                "gamma": np.asarray(inputs["gamma"], np.float32).reshape(
                    128, 1
                ),
                "beta": np.asarray(inputs["beta"], np.float32).reshape(
                    128, 1
                ),
                "iota": iota_np,
                "eye": eye_np,
            }
        )
    return in_maps

# ------------------------------------------------------------------- driver
_CACHE = {}


def kernel(x, edge_index, W1, b1, gamma, beta, W2, b2, Wl, bl):
    x = np.asarray(x, np.float32)
    edge_index = np.asarray(edge_index)
    W1 = np.asarray(W1, np.float32)
    W2 = np.asarray(W2, np.float32)

    dinv, T, streams = _preprocess(edge_index)

    NR1 = _env("NR1", "3")
    NR2 = _env("NR2", "3")
    PH = _env("PH", "9")
    key = (T.tobytes(), NR1, NR2, PH, GCH, GQS, MB, GBUFS, MBUFS, PABUFS, PTBUFS, YSACT, NOAG, NOMM, NOGATHER, NOMGEN, NOEPI, NOGEMM)
    if key not in _CACHE:
        _CACHE[key] = _build(T, NR1, NR2, PH)
    nc = _CACHE[key]

    in_maps = _make_inmaps(
        dict(x=x, W1=W1, b1=b1, gamma=gamma, beta=beta, W2=W2, b2=b2,
             Wl=Wl, bl=bl), dinv, streams)

    res = run_bass_kernel_spmd(nc, in_maps, core_ids=list(range(NCORE)))
    kernel.last_results = res

    out = np.empty((N, OUT_F), np.float32)
    for c in range(NCORE):
        lo = c * SH
        hi = min((c + 1) * SH, N)
        out[lo:hi] = res.results[c]["out"].T[: hi - lo]
    return out


# ------------------------------------------------------------------ timing
def bench(iters=8, **inputs):
    """Wall-clock the NEFF execution via the PJRT path.

    The axon tunnel adds ~80-95 ms of pure round-trip latency to every
    blocking dispatch, so a block-per-call measurement times the network,
    not the kernel.  Instead each trial submits PIPE_B executions
    back-to-back (device executes them serially), blocks once, and
    reports total/PIPE_B — steady-state per-execution time.  Returns a
    list of per-call times (min over trials is the reported number)."""
    import time
    import jax
    from jax.sharding import Mesh, PartitionSpec, NamedSharding
    from jax.experimental.shard_map import shard_map
    from concourse import bass2jax
    import concourse.mybir as mybir_

    x = np.asarray(inputs["x"], np.float32)
    edge_index = np.asarray(inputs["edge_index"])
    dinv, T, streams = _preprocess(edge_index)
    NR1 = _env("NR1", "3")
    NR2 = _env("NR2", "3")
    PH = _env("PH", "9")
    key = (T.tobytes(), NR1, NR2, PH, GCH, GQS, MB, GBUFS, MBUFS, PABUFS, PTBUFS, YSACT, NOAG, NOMM, NOGATHER, NOMGEN, NOEPI, NOGEMM)
    if key not in _CACHE:
        _CACHE[key] = _build(T, NR1, NR2, PH)
    nc = _CACHE[key]
    in_maps = _make_inmaps(inputs, dinv, streams)

    bass2jax.install_neuronx_cc_hook()
    partition_name = (
        nc.partition_id_tensor.name if nc.partition_id_tensor else None
    )
    in_names, out_names, out_avals, zero_outs = [], [], [], []
    for alloc in nc.m.functions[0].allocations:
        if not isinstance(alloc, mybir_.MemoryLocationSet):
            continue
        name = alloc.memorylocations[0].name
        if alloc.kind == "ExternalInput":
            if name != partition_name:
                in_names.append(name)
        elif alloc.kind == "ExternalOutput":
            shape = tuple(alloc.tensor_shape)
            dtype = mybir_.dt.np(alloc.dtype)
            out_avals.append(jax.core.ShapedArray(shape, dtype))
            zero_outs.append(np.zeros(shape, dtype))
            out_names.append(name)
    n_params = len(in_names)
    n_outs = len(out_avals)
    in_names.extend(out_names)
    if partition_name is not None:
        in_names.append(partition_name)

    def _body(*args):
        operands = list(args)
        if partition_name is not None:
            operands.append(bass2jax.partition_id_tensor())
        outs = bass2jax._bass_exec_p.bind(
            *operands,
            out_avals=tuple(out_avals),
            in_names=tuple(in_names),
            out_names=tuple(out_names),
            lowering_input_output_aliases=(),
            sim_require_finite=True,
            sim_require_nnan=True,
            nc=nc,
        )
        return tuple(outs)

    devices = jax.devices()[:NCORE]
    mesh = Mesh(np.asarray(devices), ("core",))
    in_specs = (PartitionSpec("core"),) * (n_params + n_outs)
    out_specs = (PartitionSpec("core"),) * len(out_names)
    sharded = jax.jit(
        shard_map(
            _body, mesh=mesh, in_specs=in_specs, out_specs=out_specs,
            check_rep=False,
        ),
        keep_unused=True,
    )
    per_core = [
        [np.asarray(m[name]) for name in in_names[:n_params]] for m in in_maps
    ]
    sh = NamedSharding(mesh, PartitionSpec("core"))
    concat_in = [
        jax.device_put(
            np.concatenate([per_core[c][i] for c in range(NCORE)], axis=0), sh
        )
        for i in range(n_params)
    ]
    concat_zeros = [
        jax.device_put(
            np.zeros((NCORE * z.shape[0], *z.shape[1:]), z.dtype), sh
        )
        for z in zero_outs
    ]

    PIPE_B = _env("PIPEB", "10")
    # warm-up: one full pipelined trial (first dispatch pays executable
    # load + tunnel setup)
    jax.block_until_ready(sharded(*concat_in, *concat_zeros))
    times = []
    for it in range(iters):
        t0 = time.perf_counter()
        outs = [sharded(*concat_in, *concat_zeros) for _ in range(PIPE_B)]
        jax.block_until_ready(outs)
        times.append((time.perf_counter() - t0) / PIPE_B)
    return times

